# revision 1
# baseline (speedup 1.0000x reference)
"""CondConv2D Trainium2 kernel.

Problem (hardcoded shapes): B=16, C_in=64, H=W=256, E=4, C_out=64, 3x3 conv,
stride=1, dilation=1, padding=1.

Sharding: data-parallel over batch. 8 cores x 2 images each. Expert weights
and routing fc params replicated (host pre-transposed for layout only).

Per-core kernel (single pass over HBM):
  - Each image resident in SBUF as 10 tiles [128, 13, 258]: partitions 0-63
    hold channels of the TOP half rows (-1..128, incl. zero pad row), 64-127
    the BOTTOM half rows (127..256). 258 = 256 + 2 zero pad cols.
  - Routing: per-tile reduce (DVE) -> pooled sums; fc via elementwise mul +
    ones-matmul (contracts the 128 partitions and broadcasts the logits to
    all partitions); + bias; sigmoid (ACT) -> r [128, 4].
  - Mixed kernels: W_mix[c_in, tap*64+c_out] = sum_e r_e * W[e,...] on DVE.
    Both partition halves hold identical copies.
  - Conv: per output row-pair, 9 taps as fp32r matmuls (K=64 c_in,
    M=64 c_out, N=512 = 2 rows x 256 px) accumulated in PSUM. Shifted-view
    rhs APs give the im2col for free. tile_position packs top-half and
    bottom-half matmuls into disjoint PE array quadrants so they run
    concurrently (NPX=2 adds a second pixel-tile pair for 4-way packing).
"""
import sys

if "/opt/trn_rl_repo" not in sys.path:
    sys.path.insert(0, "/opt/trn_rl_repo")

import numpy as np

import concourse.bacc as bacc
import concourse.mybir as mybir
import concourse.tile as tile
from concourse.bass_utils import run_bass_kernel_spmd

F32 = mybir.dt.float32
F32R = mybir.dt.float32r
BF16 = mybir.dt.bfloat16
AF = mybir.ActivationFunctionType
ALU = mybir.AluOpType

N_CORES = 8
IMGS_PER_CORE = 2
C_IN = 64
C_OUT = 64
H = 256
W = 256
E = 4
NTAP = 9
ROWS_PER_TILE = 13
N_TILES = 10          # 130 lines per half
HALF = 128            # output rows per half

NPX = 2               # pixel-row-pairs processed concurrently (1 or 2)
IMG_BUFS = 12         # resident tile slots (10 = one full image)
PSUM_BUFS = 6
STAGE_ROWS = 16       # output rows per half per staging tile


def build_nc(npx=NPX):
    nc = bacc.Bacc("TRN2", target_bir_lowering=False, debug=False,
                   num_devices=N_CORES)
    x = nc.dram_tensor("x", [IMGS_PER_CORE, C_IN, H, W], F32,
                       kind="ExternalInput")
    wt = nc.dram_tensor("wt", [128, E * NTAP * C_OUT], F32,
                        kind="ExternalInput")
    fcw = nc.dram_tensor("fcw", [128, E], F32, kind="ExternalInput")
    fcb = nc.dram_tensor("fcb", [128, E], F32, kind="ExternalInput")
    ones = nc.dram_tensor("ones", [128, 128], F32, kind="ExternalInput")
    y = nc.dram_tensor("y", [IMGS_PER_CORE, C_OUT, H, W], F32,
                       kind="ExternalOutput")

    with tile.TileContext(nc) as tc:
        with (
            tc.tile_pool(name="consts", bufs=1) as consts,
            tc.tile_pool(name="img", bufs=IMG_BUFS) as img_pool,
            tc.tile_pool(name="small", bufs=2) as small,
            tc.tile_pool(name="mix", bufs=2) as mix_pool,
            tc.tile_pool(name="stage", bufs=2) as stage_pool,
            tc.tile_pool(name="psum", bufs=PSUM_BUFS, space="PSUM") as psum_pool,
        ):
            wtt = consts.tile([128, E * NTAP * C_OUT], BF16)
            fcwt = consts.tile([128, E], F32)
            fcbt = consts.tile([128, E], F32)
            onest = consts.tile([128, 128], F32)
            nc.gpsimd.dma_start(wtt[:], wt[:])
            nc.sync.dma_start(fcwt[:], fcw[:])
            nc.sync.dma_start(fcbt[:], fcb[:])
            nc.sync.dma_start(onest[:], ones[:])

            for i in range(IMGS_PER_CORE):
                # ---- load resident tiles + per-tile pooling reduces ----
                xt = []
                partial = small.tile([128, N_TILES], F32)
                for t in range(N_TILES):
                    # unpadded, contiguous per-partition: the f32->bf16 cast
                    # DMA (SWDGE) then needs only one descriptor per
                    # partition. Edge-column handling moved into the conv
                    # matmuls (shifted psum windows).
                    xtile = img_pool.tile([128, ROWS_PER_TILE, W], BF16)
                    xt.append(xtile)
                    # top half: global rows 13t-1 .. 13t+11
                    g0 = 13 * t - 1
                    if t == 0:
                        # zero row -1; overlap line 1 (DMA rewrites it) so
                        # Tile orders memset before the DMA.
                        nc.vector.memset(xtile[0:64, 0:2, :], 0.0)
                        nc.gpsimd.dma_start(xtile[0:64, 1:13, :],
                                            x[i, :, 0:12, :])
                    else:
                        nc.gpsimd.dma_start(xtile[0:64, :, :],
                                            x[i, :, g0:g0 + 13, :])
                    # bottom half: global rows 127+13t .. 139+13t
                    b0 = 127 + 13 * t
                    if t == N_TILES - 1:
                        nc.vector.memset(xtile[64:128, 11:13, :], 0.0)
                        nc.gpsimd.dma_start(xtile[64:128, 0:12, :],
                                            x[i, :, b0:b0 + 12, :])
                    else:
                        nc.gpsimd.dma_start(xtile[64:128, :, :],
                                            x[i, :, b0:b0 + 13, :])
                    # pooling partial sums (pad cols/rows are zero).
                    # bottom tile 0 lines 0,1 = rows 127,128 already counted
                    # in the top half -> exclude.
                    if t == 0:
                        nc.vector.reduce_sum(partial[0:64, 0:1],
                                             xtile[0:64],
                                             axis=mybir.AxisListType.XY)
                        nc.vector.reduce_sum(partial[64:128, 0:1],
                                             xtile[64:128, 2:13, :],
                                             axis=mybir.AxisListType.XY)
                    else:
                        nc.vector.reduce_sum(partial[:, t:t + 1],
                                             xtile[:],
                                             axis=mybir.AxisListType.XY)

                # ---- routing ----
                pooled = small.tile([128, 1], F32)
                nc.vector.reduce_sum(pooled[:], partial[:],
                                     axis=mybir.AxisListType.X)
                tmp4 = small.tile([128, E], F32)
                nc.vector.tensor_scalar(tmp4[:], fcwt[:], pooled[:, 0:1],
                                        1.0 / float(H * W),
                                        op0=ALU.mult, op1=ALU.mult)
                ps4 = psum_pool.tile([128, E], F32, bufs=1)
                nc.tensor.matmul(ps4[:], onest[:], tmp4[:], start=True,
                                 stop=True)
                logits = small.tile([128, E], F32)
                nc.vector.tensor_tensor(logits[:], ps4[:], fcbt[:], op=ALU.add)
                rt = small.tile([128, E], F32)
                nc.scalar.activation(rt[:], logits[:], AF.Sigmoid)

                # ---- mix expert kernels ----
                wmix = mix_pool.tile([128, NTAP * C_OUT], BF16)
                wtmp = mix_pool.tile([128, NTAP * C_OUT], BF16)
                S = NTAP * C_OUT
                nc.vector.tensor_scalar_mul(wmix[:], wtt[:, 0:S], rt[:, 0:1])
                for e in range(1, E):
                    nc.vector.tensor_scalar_mul(wtmp[:], wtt[:, e * S:(e + 1) * S],
                                                rt[:, e:e + 1])
                    nc.vector.tensor_tensor(wmix[:], wmix[:], wtmp[:], op=ALU.add)

                # ---- conv ----
                n_pairs = HALF // 2                     # 64 row-pairs per half
                groups_per_stage = STAGE_ROWS // (2 * npx)
                stage = None
                for g in range(n_pairs // npx):
                    if g % groups_per_stage == 0:
                        stage = stage_pool.tile([128, STAGE_ROWS, W], F32)
                    ps_tiles = [psum_pool.tile([128, 2, W], F32, name="ps",
                                               tag="ps")
                                for _ in range(npx)]
                    # Per-pair tap order constraints:
                    #  - first tap must write the full psum region with
                    #    start=True -> must be a center tap (kw=1) of an
                    #    unsplit kh (not straddling a tile boundary).
                    #  - a split kh must not be first or last.
                    tap_orders = []
                    for px in range(npx):
                        pair = npx * g + px
                        split_kh = next((kh for kh in range(3)
                                         if (2 * pair + kh) % ROWS_PER_TILE
                                         == ROWS_PER_TILE - 1), None)
                        if split_kh is None:
                            seq = [0, 1, 2]
                        else:
                            others = [kh for kh in range(3) if kh != split_kh]
                            seq = [others[0], split_kh, others[1]]
                        order = [seq[0] * 3 + 1, seq[0] * 3 + 0, seq[0] * 3 + 2]
                        for kh in seq[1:]:
                            order += [kh * 3 + 0, kh * 3 + 1, kh * 3 + 2]
                        tap_orders.append(order)
                    for r in range(NTAP):
                        st = r == 0
                        sp = r == NTAP - 1
                        for px in range(npx):
                            pair = npx * g + px
                            tap = tap_orders[px][r]
                            kh, kw = divmod(tap, 3)
                            L = 2 * pair + kh
                            ps = ps_tiles[px]
                            # x col window <- out col window (dx = kw-1)
                            if kw == 0:
                                xs, xe, os0, oe = 0, W - 1, 1, W
                            elif kw == 1:
                                xs, xe, os0, oe = 0, W, 0, W
                            else:
                                xs, xe, os0, oe = 1, W, 0, W - 1
                            t, m = divmod(L, ROWS_PER_TILE)
                            unsplit = m <= ROWS_PER_TILE - 2
                            for half in range(2):
                                hs = slice(0, 64) if half == 0 else slice(64, 128)
                                lhsT = wmix[hs, tap * 64:(tap + 1) * 64]
                                if px == 0:
                                    tp = (0, 0) if half == 0 else (64, 64)
                                    osl = hs
                                else:
                                    tp = (0, 64) if half == 0 else (64, 0)
                                    osl = slice(64, 128) if half == 0 else slice(0, 64)
                                if kw == 1 and unsplit:
                                    # single N=512 matmul over both rows
                                    rhs = xt[t][hs, m:m + 2, :]
                                    rhs = rhs.rearrange("p a b -> p (a b)")
                                    out = ps[osl].rearrange("p a b -> p (a b)")
                                    nc.tensor.matmul(out, lhsT, rhs,
                                                     start=st, stop=sp,
                                                     tile_position=tp,
                                                     skip_group_check=True)
                                else:
                                    # one matmul per output row (2D APs)
                                    for j in range(2):
                                        tj, mj = divmod(L + j, ROWS_PER_TILE)
                                        rhs = xt[tj][hs, mj, xs:xe]
                                        nc.tensor.matmul(ps[osl, j, os0:oe],
                                                         lhsT, rhs,
                                                         start=st, stop=sp,
                                                         tile_position=tp,
                                                         skip_group_check=True)
                    # drain psum -> staging
                    r0 = (g % groups_per_stage) * 2 * npx
                    for px in range(npx):
                        dst = stage[:, r0 + 2 * px:r0 + 2 * px + 2, :]
                        src = ps_tiles[px][:]
                        if (g + px) % 2 == 0:
                            nc.vector.tensor_copy(dst, src)
                        else:
                            nc.scalar.copy(dst, src)
                    # staging full -> DMA out
                    if (g + 1) % groups_per_stage == 0:
                        mrow = (g // groups_per_stage) * STAGE_ROWS
                        if npx == 1:
                            nc.scalar.dma_start(y[i, :, mrow:mrow + STAGE_ROWS, :],
                                                stage[0:64])
                            nc.scalar.dma_start(
                                y[i, :, HALF + mrow:HALF + mrow + STAGE_ROWS, :],
                                stage[64:128])
                        else:
                            # stage blocks alternate psumA/psumB:
                            #  [0:64]   A: top (4j,4j+1)   B: bottom (4j+2,4j+3)
                            #  [64:128] A: bottom (4j,4j+1) B: top (4j+2,4j+3)
                            nj = STAGE_ROWS // 4
                            sv = stage.rearrange("p (j b r) w -> p j b r w",
                                                 j=nj, b=2, r=2)
                            ys = y[i].rearrange("c (blk four) w -> c blk four w",
                                                four=4)
                            # top rows from A blocks: rows mrow+4j+{0,1}
                            nc.scalar.dma_start(
                                ys[:, mrow // 4:mrow // 4 + nj, 0:2, :],
                                sv[0:64, :, 0, :, :])
                            # bottom rows from B blocks: rows 128+mrow+4j+{2,3}
                            nc.scalar.dma_start(
                                ys[:, (HALF + mrow) // 4:(HALF + mrow) // 4 + nj, 2:4, :],
                                sv[0:64, :, 1, :, :])
                            # bottom rows from A blocks: rows 128+mrow+4j+{0,1}
                            nc.scalar.dma_start(
                                ys[:, (HALF + mrow) // 4:(HALF + mrow) // 4 + nj, 0:2, :],
                                sv[64:128, :, 0, :, :])
                            # top rows from B blocks: rows mrow+4j+{2,3}
                            nc.scalar.dma_start(
                                ys[:, mrow // 4:mrow // 4 + nj, 2:4, :],
                                sv[64:128, :, 1, :, :])
    nc.compile()
    return nc


_NC_CACHE = {}


def _get_nc(npx=NPX):
    if npx not in _NC_CACHE:
        _NC_CACHE[npx] = build_nc(npx)
    return _NC_CACHE[npx]


def _prep_shared(weight, fc_w, fc_b):
    # [E, O, I, KH, KW] -> [I, E, KH, KW, O] -> [64, E*9*64], dup halves
    wt = np.ascontiguousarray(weight.transpose(2, 0, 3, 4, 1)).reshape(
        C_IN, E * NTAP * C_OUT)
    wt = np.concatenate([wt, wt], axis=0).astype(np.float32)
    fcw = np.concatenate([fc_w.T, fc_w.T], axis=0).astype(np.float32)
    fcb = np.tile(fc_b.reshape(1, E), (128, 1)).astype(np.float32)
    ones = np.ones((128, 128), np.float32)
    return wt, fcw, fcb, ones


def kernel(inputs, weight, fc_w, fc_b, stride=1, dilation=1, padding=1,
           _trace=False, _npx=NPX):
    assert int(stride) == 1 and int(dilation) == 1 and int(padding) == 1
    inputs = np.asarray(inputs, dtype=np.float32)
    B = inputs.shape[0]
    assert B == N_CORES * IMGS_PER_CORE
    wt, fcw, fcb, ones = _prep_shared(np.asarray(weight), np.asarray(fc_w),
                                      np.asarray(fc_b))
    nc = _get_nc(_npx)
    in_maps = []
    for c in range(N_CORES):
        in_maps.append({
            "x": np.ascontiguousarray(inputs[2 * c:2 * c + 2]),
            "wt": wt, "fcw": fcw, "fcb": fcb, "ones": ones,
        })
    res = run_bass_kernel_spmd(nc, in_maps, core_ids=list(range(N_CORES)),
                               trace=_trace)
    out = np.concatenate([res.results[c]["y"] for c in range(N_CORES)], axis=0)
    if _trace:
        return out, res
    return out



# revision 3
# speedup vs baseline: 1.2359x; 1.2359x over previous
"""CondConv2D Trainium2 kernel (v2 — bf16 I/O, monolithic padded tiles).

Problem (hardcoded shapes): B=16, C_in=64, H=W=256, E=4, C_out=64, 3x3 conv,
stride=1, dilation=1, padding=1.

Sharding: data-parallel over batch. 8 cores x 2 images each. Expert weights
and routing fc params replicated.

Key structure (vs the f32 baseline):
  - Host pre-pads each image to 258x258 (zero ring) and casts to bf16, then
    stacks the two 130-row halves (rows -1..128 / 127..256) into a flat
    128-partition layout [2*64, 130, 258]. Every input DMA is then a plain
    HWDGE transfer with one contiguous ~6.7KB descriptor per partition.
  - Each image is ONE resident SBUF tile [128, 130, 258] bf16. Input DMA is
    chunked (10 x 13 lines) so the routing pooling reduces pipeline with the
    loads; Tile's range tracking gives the conv matmuls per-line deps.
  - Routing: per-chunk reduce (DVE) -> pooled sums; fc via elementwise mul +
    ones-matmul broadcast; + bias; sigmoid (ACT) -> r [128, 4].
  - Mixed kernels: W_mix[c_in, tap*64+c_out] = sum_e r_e * W[e,...] on DVE,
    identical copies in both partition halves.
  - Conv: 2 pixel streams (px0 = row-pairs 0..31, px1 = row-pairs 32..63 of
    each half), 9 taps as bf16 matmuls (K=64, M=64, N=512 = 2 rows x 256)
    accumulated in PSUM; tile_position packs the 4 (half x px) streams into
    the 4 disjoint 64x64 PE quadrants. All taps are uniform N=512 thanks to
    the padded tile (2D access patterns give im2col for free).
  - Output: psum -> bf16 stage tiles (DVE/ACT alternate); each stage flushes
    as 2 contiguous HWDGE DMAs; host casts y back to f32.
"""
import sys

if "/opt/trn_rl_repo" not in sys.path:
    sys.path.insert(0, "/opt/trn_rl_repo")

import numpy as np

import concourse.bacc as bacc
import concourse.mybir as mybir
import concourse.tile as tile
from concourse.bass_utils import run_bass_kernel_spmd

F32 = mybir.dt.float32
BF16 = mybir.dt.bfloat16
AF = mybir.ActivationFunctionType
ALU = mybir.AluOpType

N_CORES = 8
IMGS_PER_CORE = 2
C_IN = 64
C_OUT = 64
H = 256
W = 256
E = 4
NTAP = 9
WP = W + 2            # padded width
LINES = 130           # lines per half: rows -1..128 (top) / 127..256 (bottom)
CHUNK = 13            # input DMA / reduce chunk (10 * 13 = 130)
N_CHUNKS = 10
PAIRS = 32            # row-pairs per pixel stream (= quarter image)
STAGE_ROWS = 16       # output rows per quarter per staging tile
PSUM_BUFS = 6

BF16_NP = mybir.dt.np(BF16)


def build_nc():
    nc = bacc.Bacc("TRN2", target_bir_lowering=False, debug=False,
                   num_devices=N_CORES)
    x = nc.dram_tensor("x", [IMGS_PER_CORE, 128, LINES, WP], BF16,
                       kind="ExternalInput")
    wt = nc.dram_tensor("wt", [128, E * NTAP * C_OUT], BF16,
                        kind="ExternalInput")
    fcw = nc.dram_tensor("fcw", [128, E], F32, kind="ExternalInput")
    fcb = nc.dram_tensor("fcb", [128, E], F32, kind="ExternalInput")
    ones = nc.dram_tensor("ones", [128, 128], F32, kind="ExternalInput")
    y = nc.dram_tensor("y", [IMGS_PER_CORE, C_OUT, H, W], BF16,
                       kind="ExternalOutput")

    gps = STAGE_ROWS // 2              # conv groups per stage block

    with tile.TileContext(nc) as tc:
        with (
            tc.tile_pool(name="consts", bufs=1) as consts,
            tc.tile_pool(name="img", bufs=IMGS_PER_CORE) as img_pool,
            tc.tile_pool(name="small", bufs=4) as small,
            tc.tile_pool(name="mix", bufs=2 * IMGS_PER_CORE) as mix_pool,
            tc.tile_pool(name="stage", bufs=2) as stage_pool,
            tc.tile_pool(name="psum", bufs=PSUM_BUFS, space="PSUM") as psum_pool,
        ):
            wtt = consts.tile([128, E * NTAP * C_OUT], BF16)
            fcwt = consts.tile([128, E], F32)
            fcbt = consts.tile([128, E], F32)
            onest = consts.tile([128, 128], F32)
            nc.sync.dma_start(wtt[:], wt[:])
            nc.sync.dma_start(fcwt[:], fcw[:])
            nc.sync.dma_start(fcbt[:], fcb[:])
            nc.sync.dma_start(onest[:], ones[:])

            for i in range(IMGS_PER_CORE):
                # ---- chunked load + pooling reduces ----
                xt = img_pool.tile([128, LINES, WP], BF16)
                partial = small.tile([128, N_CHUNKS], F32)
                for t in range(N_CHUNKS):
                    r0 = CHUNK * t
                    nc.sync.dma_start(xt[:, r0:r0 + CHUNK, :],
                                      x[i, :, r0:r0 + CHUNK, :])
                    if t == 0:
                        # bottom lines 0,1 = rows 127,128, already counted in
                        # the top half -> exclude from the sum.
                        nc.vector.reduce_sum(partial[0:64, 0:1],
                                             xt[0:64, 0:CHUNK, :],
                                             axis=mybir.AxisListType.XY)
                        nc.vector.reduce_sum(partial[64:128, 0:1],
                                             xt[64:128, 2:CHUNK, :],
                                             axis=mybir.AxisListType.XY)
                    else:
                        nc.vector.reduce_sum(partial[:, t:t + 1],
                                             xt[:, r0:r0 + CHUNK, :],
                                             axis=mybir.AxisListType.XY)

                # ---- routing ----
                pooled = small.tile([128, 1], F32)
                nc.vector.reduce_sum(pooled[:], partial[:],
                                     axis=mybir.AxisListType.X)
                tmp4 = small.tile([128, E], F32)
                nc.vector.tensor_scalar(tmp4[:], fcwt[:], pooled[:, 0:1],
                                        1.0 / float(H * W),
                                        op0=ALU.mult, op1=ALU.mult)
                ps4 = psum_pool.tile([128, E], F32, bufs=1)
                nc.tensor.matmul(ps4[:], onest[:], tmp4[:], start=True,
                                 stop=True)
                logits = small.tile([128, E], F32)
                nc.vector.tensor_tensor(logits[:], ps4[:], fcbt[:], op=ALU.add)
                rt = small.tile([128, E], F32)
                nc.scalar.activation(rt[:], logits[:], AF.Sigmoid)

                # ---- mix expert kernels ----
                S = NTAP * C_OUT
                wmix = mix_pool.tile([128, S], BF16)
                wtmp = mix_pool.tile([128, S], BF16)
                nc.vector.tensor_scalar_mul(wmix[:], wtt[:, 0:S], rt[:, 0:1])
                for e in range(1, E):
                    nc.vector.tensor_scalar_mul(wtmp[:], wtt[:, e * S:(e + 1) * S],
                                                rt[:, e:e + 1])
                    nc.vector.tensor_tensor(wmix[:], wmix[:], wtmp[:], op=ALU.add)

                # ---- conv ----
                # Quadrant map per group g (pair p0 = g, pair p1 = 32+g):
                #   psA[0:64]   <- top    rows 2g..2g+1       (tp (0,0))
                #   psA[64:128] <- bottom rows 128+2g..       (tp (64,64))
                #   psB[64:128] <- top    rows 64+2g..        (tp (0,64))
                #   psB[0:64]   <- bottom rows 192+2g..       (tp (64,0))
                stageA = stageB = None
                for g in range(PAIRS):
                    if g % gps == 0:
                        stageA = stage_pool.tile([128, STAGE_ROWS, W], BF16)
                        stageB = stage_pool.tile([128, STAGE_ROWS, W], BF16)
                    psA = psum_pool.tile([128, 2, W], F32, name="ps", tag="ps")
                    psB = psum_pool.tile([128, 2, W], F32, name="ps", tag="ps")
                    outA = psA.rearrange("p a b -> p (a b)")
                    outB = psB.rearrange("p a b -> p (a b)")
                    lA = 2 * g
                    lB = 64 + 2 * g
                    for tap in range(NTAP):
                        kh, kw = divmod(tap, 3)
                        st = tap == 0
                        sp = tap == NTAP - 1
                        lhs_t = wmix[0:64, tap * 64:(tap + 1) * 64]
                        lhs_b = wmix[64:128, tap * 64:(tap + 1) * 64]
                        nc.tensor.matmul(
                            outA[0:64], lhs_t,
                            xt[0:64, lA + kh:lA + kh + 2, kw:kw + W],
                            start=st, stop=sp, tile_position=(0, 0),
                            skip_group_check=True)
                        nc.tensor.matmul(
                            outA[64:128], lhs_b,
                            xt[64:128, lA + kh:lA + kh + 2, kw:kw + W],
                            start=st, stop=sp, tile_position=(64, 64),
                            skip_group_check=True)
                        nc.tensor.matmul(
                            outB[64:128], lhs_t,
                            xt[0:64, lB + kh:lB + kh + 2, kw:kw + W],
                            start=st, stop=sp, tile_position=(0, 64),
                            skip_group_check=True)
                        nc.tensor.matmul(
                            outB[0:64], lhs_b,
                            xt[64:128, lB + kh:lB + kh + 2, kw:kw + W],
                            start=st, stop=sp, tile_position=(64, 0),
                            skip_group_check=True)
                    # drain psum -> bf16 staging
                    r0 = (g % gps) * 2
                    if g % 2 == 0:
                        nc.vector.tensor_copy(stageA[:, r0:r0 + 2, :], psA[:])
                        nc.scalar.copy(stageB[:, r0:r0 + 2, :], psB[:])
                    else:
                        nc.scalar.copy(stageA[:, r0:r0 + 2, :], psA[:])
                        nc.vector.tensor_copy(stageB[:, r0:r0 + 2, :], psB[:])
                    # stage full -> 4 contiguous DMAs (one per quarter)
                    if (g + 1) % gps == 0:
                        base = (g // gps) * STAGE_ROWS
                        nc.scalar.dma_start(
                            y[i, :, base:base + STAGE_ROWS, :], stageA[0:64])
                        nc.scalar.dma_start(
                            y[i, :, 128 + base:128 + base + STAGE_ROWS, :],
                            stageA[64:128])
                        nc.scalar.dma_start(
                            y[i, :, 192 + base:192 + base + STAGE_ROWS, :],
                            stageB[0:64])
                        nc.scalar.dma_start(
                            y[i, :, 64 + base:64 + base + STAGE_ROWS, :],
                            stageB[64:128])
    nc.compile()
    return nc


_NC_CACHE = {}


def _get_nc():
    if "nc" not in _NC_CACHE:
        _NC_CACHE["nc"] = build_nc()
    return _NC_CACHE["nc"]


def _prep_shared(weight, fc_w, fc_b):
    # [E, O, I, KH, KW] -> [I, E, KH, KW, O] -> [64, E*9*64], dup halves
    wt = np.ascontiguousarray(weight.transpose(2, 0, 3, 4, 1)).reshape(
        C_IN, E * NTAP * C_OUT)
    wt = np.concatenate([wt, wt], axis=0).astype(BF16_NP)
    fcw = np.concatenate([fc_w.T, fc_w.T], axis=0).astype(np.float32)
    fcb = np.tile(fc_b.reshape(1, E), (128, 1)).astype(np.float32)
    ones = np.ones((128, 128), np.float32)
    return wt, fcw, fcb, ones


def _prep_x(inputs):
    # pad to 258x258 zero ring, cast bf16, stack halves -> [B, 128, 130, 258]
    B = inputs.shape[0]
    xp = np.zeros((B, C_IN, H + 2, W + 2), dtype=BF16_NP)
    xp[:, :, 1:H + 1, 1:W + 1] = inputs.astype(BF16_NP)
    xh = np.empty((B, 2, C_IN, LINES, WP), dtype=BF16_NP)
    xh[:, 0] = xp[:, :, 0:LINES, :]
    xh[:, 1] = xp[:, :, H - LINES + 2:H + 2, :]
    return xh.reshape(B, 2 * C_IN, LINES, WP)


def kernel(inputs, weight, fc_w, fc_b, stride=1, dilation=1, padding=1,
           _trace=False):
    assert int(stride) == 1 and int(dilation) == 1 and int(padding) == 1
    inputs = np.asarray(inputs, dtype=np.float32)
    B = inputs.shape[0]
    assert B == N_CORES * IMGS_PER_CORE
    wt, fcw, fcb, ones = _prep_shared(np.asarray(weight), np.asarray(fc_w),
                                      np.asarray(fc_b))
    xh = _prep_x(inputs)
    nc = _get_nc()
    in_maps = []
    for c in range(N_CORES):
        in_maps.append({
            "x": np.ascontiguousarray(xh[2 * c:2 * c + 2]),
            "wt": wt, "fcw": fcw, "fcb": fcb, "ones": ones,
        })
    res = run_bass_kernel_spmd(nc, in_maps, core_ids=list(range(N_CORES)),
                               trace=_trace)
    out = np.concatenate(
        [res.results[c]["y"].astype(np.float32) for c in range(N_CORES)],
        axis=0)
    if _trace:
        return out, res
    return out


# revision 7
# speedup vs baseline: 1.3596x; 1.1001x over previous
"""CondConv2D Trainium2 kernel (v3).

Problem (hardcoded shapes): B=16, C_in=64, H=W=256, E=4, C_out=64, 3x3 conv,
stride=1, dilation=1, padding=1.

Sharding: data-parallel over batch. 8 cores x 2 images each. Expert weights
and routing fc params replicated.

Structure:
  - Host pre-pads each image to 258x258 (zero ring), casts to bf16, stacks
    the two 130-line halves (rows -1..128 / 127..256) into [128, 130, 258]
    (partition = half*64 + c_in). Input DMAs are plain HWDGE chunks with one
    contiguous ~6.7KB descriptor per partition.
  - Each image is ONE resident SBUF tile. Chunked loads pipeline with the
    routing pooling, which runs as DVE tensor_scalar(identity) + accum_out
    (4x bf16 perf mode; tensor_reduce would be 1x-only and 4x slower).
  - Routing: pooled sums -> fc via elementwise mul + ones-matmul broadcast;
    + bias; sigmoid -> r [128, 4]. Mixed kernels on DVE.
  - Conv: 4 streams (2 halves x 2 pixel streams; px0 = row-pairs 0..31, px1
    = 32..63 of each half) as bf16 matmuls K=64, M=64, N=512 accumulated in
    one 2-bank PSUM tile per group; tile_position packs the streams into the
    4 disjoint 64x64 PE quadrants. All taps uniform N=512 (padded tile gives
    im2col for free via 2D APs).
  - Output: one drain per group [128, 2, 2, 256] f32->bf16 (DVE/ACT 3:1),
    one out-DMA per 16-row stage into a quarter-permuted bf16 layout
    y2[128, 128, 256] (partition p<64: row r<64 = Q0 rows, r>=64 = Q3;
    p>=64: r<64 = Q2, r>=64 = Q1); host reassembles + casts to f32.
  - Emission order interleaves image 1's loads/reduces and routing into
    image 0's conv groups so the PE never idles between images.
"""
import sys

if "/opt/trn_rl_repo" not in sys.path:
    sys.path.insert(0, "/opt/trn_rl_repo")

import numpy as np

import concourse.bacc as bacc
import concourse.mybir as mybir
import concourse.tile as tile
from concourse.bass_utils import run_bass_kernel_spmd

F32 = mybir.dt.float32
BF16 = mybir.dt.bfloat16
AF = mybir.ActivationFunctionType
ALU = mybir.AluOpType

N_CORES = 8
IMGS_PER_CORE = 2
C_IN = 64
C_OUT = 64
H = 256
W = 256
E = 4
NTAP = 9
WP = W + 2            # padded width
LINES = 130           # lines per half
CHUNK = 13            # input DMA / reduce chunk (10 * 13 = 130)
N_CHUNKS = 10
PAIRS = 32            # row-pairs per pixel stream (= quarter image)
STAGE_ROWS = 16       # output rows per quarter per staging tile
PSUM_BUFS = 3

BF16_NP = mybir.dt.np(BF16)


def build_nc():
    nc = bacc.Bacc("TRN2", target_bir_lowering=False, debug=False,
                   num_devices=N_CORES)
    x = nc.dram_tensor("x", [IMGS_PER_CORE, 128, LINES, WP], BF16,
                       kind="ExternalInput")
    wt = nc.dram_tensor("wt", [128, E * NTAP * C_OUT], BF16,
                        kind="ExternalInput")
    fcw = nc.dram_tensor("fcw", [128, E], F32, kind="ExternalInput")
    fcb = nc.dram_tensor("fcb", [128, E], F32, kind="ExternalInput")
    ones = nc.dram_tensor("ones", [128, 128], F32, kind="ExternalInput")
    y = nc.dram_tensor("y", [IMGS_PER_CORE, 128, 128, W], BF16,
                       kind="ExternalOutput")

    gps = STAGE_ROWS // 2              # conv groups per stage block
    S = NTAP * C_OUT

    with tile.TileContext(nc) as tc:
        with (
            tc.tile_pool(name="consts", bufs=1) as consts,
            tc.tile_pool(name="img", bufs=IMGS_PER_CORE) as img_pool,
            tc.tile_pool(name="small", bufs=4) as small,
            tc.tile_pool(name="mix", bufs=2 * IMGS_PER_CORE) as mix_pool,
            tc.tile_pool(name="stage", bufs=2) as stage_pool,
            tc.tile_pool(name="psum", bufs=PSUM_BUFS, space="PSUM") as psum_pool,
        ):
            wtt = consts.tile([128, E * NTAP * C_OUT], BF16)
            fcwt = consts.tile([128, E], F32)
            fcbt = consts.tile([128, E], F32)
            onest = consts.tile([128, 128], F32)
            # consts ride the (idle-early) scalar HWDGE ring
            nc.scalar.dma_start(wtt[:], wt[:])
            nc.scalar.dma_start(fcwt[:], fcw[:])
            nc.scalar.dma_start(fcbt[:], fcb[:])
            nc.scalar.dma_start(onest[:], ones[:])

            xts = [img_pool.tile([128, LINES, WP], BF16, name=f"xt{i}",
                                 tag="xt")
                   for i in range(IMGS_PER_CORE)]
            partials = [small.tile([128, N_CHUNKS], F32, name=f"partial{i}",
                                   tag="partial")
                        for i in range(IMGS_PER_CORE)]

            def load_chunk(i, t):
                """DMA chunk t of image i and accumulate its pooling sums."""
                xt, partial = xts[i], partials[i]
                r0 = CHUNK * t
                nc.sync.dma_start(xt[:, r0:r0 + CHUNK, :],
                                  x[i, :, r0:r0 + CHUNK, :])
                if t == 0:
                    # bottom lines 0,1 = rows 127,128 already counted in the
                    # top half -> exclude from the bottom sum.
                    top = xt[0:64, 0:CHUNK, :].rearrange("p a b -> p (a b)")
                    bot = xt[64:128, 2:CHUNK, :].rearrange("p a b -> p (a b)")
                    nc.vector.tensor_scalar(top, top, 1.0, None, op0=ALU.mult,
                                            op1=ALU.add,
                                            accum_out=partial[0:64, 0:1])
                    nc.vector.tensor_scalar(bot, bot, 1.0, None, op0=ALU.mult,
                                            op1=ALU.add,
                                            accum_out=partial[64:128, 0:1])
                else:
                    fl = xt[:, r0:r0 + CHUNK, :].rearrange("p a b -> p (a b)")
                    nc.vector.tensor_scalar(fl, fl, 1.0, None, op0=ALU.mult,
                                            op1=ALU.add,
                                            accum_out=partial[:, t:t + 1])

            def routing_and_mix(i):
                partial = partials[i]
                pooled = small.tile([128, 1], F32)
                nc.vector.reduce_sum(pooled[:], partial[:],
                                     axis=mybir.AxisListType.X)
                tmp4 = small.tile([128, E], F32)
                nc.vector.tensor_scalar(tmp4[:], fcwt[:], pooled[:, 0:1],
                                        1.0 / float(H * W),
                                        op0=ALU.mult, op1=ALU.mult)
                ps4 = psum_pool.tile([128, E], F32, bufs=1)
                nc.tensor.matmul(ps4[:], onest[:], tmp4[:], start=True,
                                 stop=True)
                logits = small.tile([128, E], F32)
                nc.vector.tensor_tensor(logits[:], ps4[:], fcbt[:], op=ALU.add)
                rt = small.tile([128, E], F32)
                nc.scalar.activation(rt[:], logits[:], AF.Sigmoid)
                wmix = mix_pool.tile([128, S], BF16)
                wtmp = mix_pool.tile([128, S], BF16)
                nc.vector.tensor_scalar_mul(wmix[:], wtt[:, 0:S], rt[:, 0:1])
                for e in range(1, E):
                    nc.vector.tensor_scalar_mul(wtmp[:],
                                                wtt[:, e * S:(e + 1) * S],
                                                rt[:, e:e + 1])
                    nc.vector.tensor_tensor(wmix[:], wmix[:], wtmp[:],
                                            op=ALU.add)
                return wmix

            def conv_group(i, g, wmix, stage):
                """One group: pairs (g, 32+g) of both halves, 9 taps."""
                xt = xts[i]
                ps = psum_pool.tile([128, 2, 2, W], F32, name="ps", tag="ps")
                outA = ps[:, 0].rearrange("p a b -> p (a b)")
                outB = ps[:, 1].rearrange("p a b -> p (a b)")
                lA = 2 * g
                lB = 64 + 2 * g
                for tap in range(NTAP):
                    kh, kw = divmod(tap, 3)
                    st = tap == 0
                    sp = tap == NTAP - 1
                    lhs_t = wmix[0:64, tap * 64:(tap + 1) * 64]
                    lhs_b = wmix[64:128, tap * 64:(tap + 1) * 64]
                    nc.tensor.matmul(
                        outA[0:64], lhs_t,
                        xt[0:64, lA + kh:lA + kh + 2, kw:kw + W],
                        start=st, stop=sp, tile_position=(0, 0),
                        skip_group_check=True)
                    nc.tensor.matmul(
                        outA[64:128], lhs_b,
                        xt[64:128, lA + kh:lA + kh + 2, kw:kw + W],
                        start=st, stop=sp, tile_position=(64, 64),
                        skip_group_check=True)
                    nc.tensor.matmul(
                        outB[64:128], lhs_t,
                        xt[0:64, lB + kh:lB + kh + 2, kw:kw + W],
                        start=st, stop=sp, tile_position=(0, 64),
                        skip_group_check=True)
                    nc.tensor.matmul(
                        outB[0:64], lhs_b,
                        xt[64:128, lB + kh:lB + kh + 2, kw:kw + W],
                        start=st, stop=sp, tile_position=(64, 0),
                        skip_group_check=True)
                # drain psum -> bf16 staging (DVE:ACT = 3:1)
                r0 = (g % gps) * 2
                dst = stage[:, :, r0:r0 + 2, :]
                if g % 4 == 3:
                    nc.scalar.copy(dst, ps[:])
                else:
                    nc.vector.tensor_copy(dst, ps[:])
                # stage full -> one out-DMA (rows base.. of both stream rows)
                if (g + 1) % gps == 0:
                    base = (g // gps) * STAGE_ROWS
                    dst = y[i].rearrange("p (s r) w -> p s r w", s=2)
                    nc.scalar.dma_start(
                        dst[:, :, base:base + STAGE_ROWS, :], stage[:])

            # ---- image 0 load + routing ----
            for t in range(N_CHUNKS):
                load_chunk(0, t)
            wmix0 = routing_and_mix(0)

            # ---- conv image 0, interleaving image 1 prefetch ----
            wmix1 = None
            stage = None
            for g in range(PAIRS):
                if g % gps == 0:
                    stage = stage_pool.tile([128, 2, STAGE_ROWS, W], BF16)
                if g < 2 * N_CHUNKS and g % 2 == 0:
                    load_chunk(1, g // 2)
                conv_group(0, g, wmix0, stage)
                if g == 26:
                    wmix1 = routing_and_mix(1)

            # ---- conv image 1 ----
            for g in range(PAIRS):
                if g % gps == 0:
                    stage = stage_pool.tile([128, 2, STAGE_ROWS, W], BF16)
                conv_group(1, g, wmix1, stage)
    nc.compile()
    return nc


_NC_CACHE = {}


def _get_nc():
    if "nc" not in _NC_CACHE:
        _NC_CACHE["nc"] = build_nc()
    return _NC_CACHE["nc"]


def _prep_shared(weight, fc_w, fc_b):
    # [E, O, I, KH, KW] -> [I, E, KH, KW, O] -> [64, E*9*64], dup halves
    wt = np.ascontiguousarray(weight.transpose(2, 0, 3, 4, 1)).reshape(
        C_IN, E * NTAP * C_OUT)
    wt = np.concatenate([wt, wt], axis=0).astype(BF16_NP)
    fcw = np.concatenate([fc_w.T, fc_w.T], axis=0).astype(np.float32)
    fcb = np.tile(fc_b.reshape(1, E), (128, 1)).astype(np.float32)
    ones = np.ones((128, 128), np.float32)
    return wt, fcw, fcb, ones


def _prep_x(inputs):
    # pad to 258x258 zero ring, cast bf16, stack halves -> [B, 128, 130, 258]
    B = inputs.shape[0]
    xp = np.zeros((B, C_IN, H + 2, W + 2), dtype=BF16_NP)
    xp[:, :, 1:H + 1, 1:W + 1] = inputs.astype(BF16_NP)
    xh = np.empty((B, 2, C_IN, LINES, WP), dtype=BF16_NP)
    xh[:, 0] = xp[:, :, 0:LINES, :]
    xh[:, 1] = xp[:, :, H - LINES + 2:H + 2, :]
    return xh.reshape(B, 2 * C_IN, LINES, WP)


def _unpack_y(y2):
    # y2: [IMGS, 128, 128, W] quarter-permuted -> [IMGS, C_OUT, H, W] f32
    out = np.empty((y2.shape[0], C_OUT, H, W), dtype=np.float32)
    out[:, :, 0:64] = y2[:, 0:64, 0:64]
    out[:, :, 64:128] = y2[:, 64:128, 64:128]
    out[:, :, 128:192] = y2[:, 64:128, 0:64]
    out[:, :, 192:256] = y2[:, 0:64, 64:128]
    return out


def kernel(inputs, weight, fc_w, fc_b, stride=1, dilation=1, padding=1,
           _trace=False):
    assert int(stride) == 1 and int(dilation) == 1 and int(padding) == 1
    inputs = np.asarray(inputs, dtype=np.float32)
    B = inputs.shape[0]
    assert B == N_CORES * IMGS_PER_CORE
    wt, fcw, fcb, ones = _prep_shared(np.asarray(weight), np.asarray(fc_w),
                                      np.asarray(fc_b))
    xh = _prep_x(inputs)
    nc = _get_nc()
    in_maps = []
    for c in range(N_CORES):
        in_maps.append({
            "x": np.ascontiguousarray(xh[2 * c:2 * c + 2]),
            "wt": wt, "fcw": fcw, "fcb": fcb, "ones": ones,
        })
    res = run_bass_kernel_spmd(nc, in_maps, core_ids=list(range(N_CORES)),
                               trace=_trace)
    out = np.concatenate(
        [_unpack_y(res.results[c]["y"]) for c in range(N_CORES)], axis=0)
    if _trace:
        return out, res
    return out


# revision 8
# speedup vs baseline: 1.6142x; 1.1873x over previous
"""CondConv2D Trainium2 kernel (v4).

Problem (hardcoded shapes): B=16, C_in=64, H=W=256, E=4, C_out=64, 3x3 conv,
stride=1, dilation=1, padding=1.

Sharding: data-parallel over batch. 8 cores x 2 images each. Expert weights
and routing fc params replicated.

Structure:
  - Host pre-pads each image to 258x258 (zero ring), casts to bf16, stacks
    the two 130-line halves (rows -1..128 / 127..256) into [128, 130, 258]
    (partition = half*64 + c_in). Input DMAs are plain HWDGE chunks with one
    contiguous ~7KB descriptor per partition; last chunk is small so the
    routing tail after the final DMA is short.
  - Each image is ONE resident SBUF tile. Chunked loads pipeline with the
    pooling, which runs as identity-op + accum_out reduces alternating
    between DVE (tensor_scalar) and ACT (activation Copy) so neither engine
    falls behind the DMA stream (tensor_reduce-style ops are 1x-only).
  - Dummy matmuls tied to image-0 chunk arrivals keep the PE HAM warm so
    conv starts at full clock.
  - Routing: pooled sums -> fc via elementwise mul + ones-matmul broadcast;
    + bias; sigmoid -> r [128, 4]. Mix via scalar_tensor_tensor chain.
  - Conv: 4 streams (2 halves x 2 pixel streams; px0 = row-pairs 0..31, px1
    = 32..63 of each half) as bf16 matmuls K=64, M=64, N=512 accumulated in
    one 2-bank PSUM tile per group; tile_position packs the streams into the
    4 disjoint 64x64 PE quadrants. All taps uniform N=512 (padded tile gives
    im2col for free via 2D APs).
  - Output: one drain per group [128, 2, 2, 256] f32->bf16 (DVE/ACT 2:1),
    one out-DMA per 16-row stage into a quarter-permuted bf16 layout
    y2[128, 128, 256]; host reassembles + casts to f32. The very last stage
    flushes in two pieces to shorten the kernel tail.
  - Emission order interleaves image 1's loads/reduces and routing into
    image 0's conv groups so the PE never idles between images.
"""
import sys

if "/opt/trn_rl_repo" not in sys.path:
    sys.path.insert(0, "/opt/trn_rl_repo")

import numpy as np

import concourse.bacc as bacc
import concourse.mybir as mybir
import concourse.tile as tile
from concourse.bass_utils import run_bass_kernel_spmd

F32 = mybir.dt.float32
BF16 = mybir.dt.bfloat16
AF = mybir.ActivationFunctionType
ALU = mybir.AluOpType

N_CORES = 8
IMGS_PER_CORE = 2
C_IN = 64
C_OUT = 64
H = 256
W = 256
E = 4
NTAP = 9
WP = W + 2            # padded width
LINES = 130           # lines per half
CHUNKS = [14] * 9 + [4]          # input DMA / reduce chunk lines
N_CHUNKS = len(CHUNKS)
OFFS = [sum(CHUNKS[:i]) for i in range(N_CHUNKS)]
PAIRS = 32            # row-pairs per pixel stream (= quarter image)
STAGE_ROWS = 16       # output rows per quarter per staging tile
PSUM_BUFS = 3

BF16_NP = mybir.dt.np(BF16)


def build_nc():
    nc = bacc.Bacc("TRN2", target_bir_lowering=False, debug=False,
                   num_devices=N_CORES)
    x = nc.dram_tensor("x", [IMGS_PER_CORE, 128, LINES, WP], BF16,
                       kind="ExternalInput")
    wt = nc.dram_tensor("wt", [128, E * NTAP * C_OUT], BF16,
                        kind="ExternalInput")
    fcw = nc.dram_tensor("fcw", [128, E], F32, kind="ExternalInput")
    fcb = nc.dram_tensor("fcb", [128, E], F32, kind="ExternalInput")
    ones = nc.dram_tensor("ones", [128, 128], F32, kind="ExternalInput")
    y = nc.dram_tensor("y", [IMGS_PER_CORE, 128, 128, W], BF16,
                       kind="ExternalOutput")

    gps = STAGE_ROWS // 2              # conv groups per stage block
    S = NTAP * C_OUT

    with tile.TileContext(nc) as tc:
        with (
            tc.tile_pool(name="consts", bufs=1) as consts,
            tc.tile_pool(name="img", bufs=IMGS_PER_CORE) as img_pool,
            tc.tile_pool(name="small", bufs=4) as small,
            tc.tile_pool(name="mix", bufs=2 * IMGS_PER_CORE) as mix_pool,
            tc.tile_pool(name="stage", bufs=2) as stage_pool,
            tc.tile_pool(name="psum", bufs=PSUM_BUFS, space="PSUM") as psum_pool,
        ):
            wtt = consts.tile([128, E * NTAP * C_OUT], BF16)
            fcwt = consts.tile([128, E], F32)
            fcbt = consts.tile([128, E], F32)
            onest = consts.tile([128, 128], F32)
            # consts ride the (idle-early) scalar HWDGE ring
            nc.scalar.dma_start(wtt[:], wt[:])
            nc.scalar.dma_start(fcwt[:], fcw[:])
            nc.scalar.dma_start(fcbt[:], fcb[:])
            nc.scalar.dma_start(onest[:], ones[:])

            xts = [img_pool.tile([128, LINES, WP], BF16, name=f"xt{i}",
                                 tag="xt")
                   for i in range(IMGS_PER_CORE)]
            partials = [small.tile([128, N_CHUNKS], F32, name=f"partial{i}",
                                   tag="partial")
                        for i in range(IMGS_PER_CORE)]

            def reduce_into(engine, ap, acc):
                """acc[:, 0:1] = sum(ap) via identity op + accumulator."""
                if engine == "v":
                    nc.vector.tensor_scalar(ap, ap, 1.0, None, op0=ALU.mult,
                                            op1=ALU.add, accum_out=acc)
                else:
                    nc.scalar.activation(ap, ap, AF.Copy, accum_out=acc)

            def load_chunk(i, t, warm=False):
                """DMA chunk t of image i and accumulate its pooling sums."""
                xt, partial = xts[i], partials[i]
                r0, ln = OFFS[t], CHUNKS[t]
                nc.sync.dma_start(xt[:, r0:r0 + ln, :],
                                  x[i, :, r0:r0 + ln, :])
                eng = "v" if t % 2 == 0 else "s"
                if t == 0:
                    # bottom lines 0,1 = rows 127,128 already counted in the
                    # top half -> exclude from the bottom sum.
                    top = xt[0:64, 0:ln, :].rearrange("p a b -> p (a b)")
                    bot = xt[64:128, 2:ln, :].rearrange("p a b -> p (a b)")
                    reduce_into("v", top, partial[0:64, 0:1])
                    reduce_into("s", bot, partial[64:128, 0:1])
                else:
                    fl = xt[:, r0:r0 + ln, :].rearrange("p a b -> p (a b)")
                    reduce_into(eng, fl, partial[:, t:t + 1])
                if warm:
                    # dummy matmul keeps the PE HAM un-throttled during the
                    # first image's load phase
                    wps = psum_pool.tile([128, 2, W], F32, name="warm",
                                         tag="warm", bufs=1)
                    nc.tensor.matmul(
                        wps[0:64].rearrange("p a b -> p (a b)"),
                        wtt[0:64, 0:64], xt[0:64, r0:r0 + 2, 1:1 + W],
                        start=True, stop=True, skip_group_check=True)

            def routing_and_mix(i):
                partial = partials[i]
                pooled = small.tile([128, 1], F32)
                nc.vector.reduce_sum(pooled[:], partial[:],
                                     axis=mybir.AxisListType.X)
                tmp4 = small.tile([128, E], F32)
                nc.vector.tensor_scalar(tmp4[:], fcwt[:], pooled[:, 0:1],
                                        1.0 / float(H * W),
                                        op0=ALU.mult, op1=ALU.mult)
                ps4 = psum_pool.tile([128, E], F32, bufs=1)
                nc.tensor.matmul(ps4[:], onest[:], tmp4[:], start=True,
                                 stop=True)
                logits = small.tile([128, E], F32)
                nc.vector.tensor_tensor(logits[:], ps4[:], fcbt[:], op=ALU.add)
                rt = small.tile([128, E], F32)
                nc.scalar.activation(rt[:], logits[:], AF.Sigmoid)
                wmix = mix_pool.tile([128, S], BF16)
                nc.vector.tensor_scalar_mul(wmix[:], wtt[:, 0:S], rt[:, 0:1])
                for e in range(1, E):
                    nc.vector.scalar_tensor_tensor(
                        wmix[:], wtt[:, e * S:(e + 1) * S], rt[:, e:e + 1],
                        wmix[:], op0=ALU.mult, op1=ALU.add)
                return wmix

            def flush(i, stage, base, lo, hi):
                dst = y[i].rearrange("p (s r) w -> p s r w", s=2)
                nc.scalar.dma_start(dst[:, :, base + lo:base + hi, :],
                                    stage[:, :, lo:hi, :])

            def conv_group(i, g, wmix, stage, last=False):
                """One group: pairs (g, 32+g) of both halves, 9 taps."""
                xt = xts[i]
                ps = psum_pool.tile([128, 2, 2, W], F32, name="ps", tag="ps")
                outA = ps[:, 0].rearrange("p a b -> p (a b)")
                outB = ps[:, 1].rearrange("p a b -> p (a b)")
                lA = 2 * g
                lB = 64 + 2 * g
                for tap in range(NTAP):
                    kh, kw = divmod(tap, 3)
                    st = tap == 0
                    sp = tap == NTAP - 1
                    lhs_t = wmix[0:64, tap * 64:(tap + 1) * 64]
                    lhs_b = wmix[64:128, tap * 64:(tap + 1) * 64]
                    nc.tensor.matmul(
                        outA[0:64], lhs_t,
                        xt[0:64, lA + kh:lA + kh + 2, kw:kw + W],
                        start=st, stop=sp, tile_position=(0, 0),
                        skip_group_check=True)
                    nc.tensor.matmul(
                        outA[64:128], lhs_b,
                        xt[64:128, lA + kh:lA + kh + 2, kw:kw + W],
                        start=st, stop=sp, tile_position=(64, 64),
                        skip_group_check=True)
                    nc.tensor.matmul(
                        outB[64:128], lhs_t,
                        xt[0:64, lB + kh:lB + kh + 2, kw:kw + W],
                        start=st, stop=sp, tile_position=(0, 64),
                        skip_group_check=True)
                    nc.tensor.matmul(
                        outB[0:64], lhs_b,
                        xt[64:128, lB + kh:lB + kh + 2, kw:kw + W],
                        start=st, stop=sp, tile_position=(64, 0),
                        skip_group_check=True)
                # drain psum -> bf16 staging (DVE:ACT = 2:1)
                r0 = (g % gps) * 2
                dst = stage[:, :, r0:r0 + 2, :]
                if g % 3 == 2:
                    nc.scalar.copy(dst, ps[:])
                else:
                    nc.vector.tensor_copy(dst, ps[:])
                # stage full -> out-DMA (split tail for the very last stage)
                base = (g // gps) * STAGE_ROWS
                if last and (g + 1) % gps == 6:
                    flush(i, stage, base, 0, 12)
                elif (g + 1) % gps == 0:
                    if last:
                        flush(i, stage, base, 12, 16)
                    else:
                        flush(i, stage, base, 0, STAGE_ROWS)

            # ---- image 0 load + routing (PE kept warm by dummy matmuls) ----
            for t in range(N_CHUNKS):
                load_chunk(0, t, warm=True)
            wmix0 = routing_and_mix(0)

            # ---- conv image 0, interleaving image 1 prefetch ----
            wmix1 = None
            stage = None
            for g in range(PAIRS):
                if g % gps == 0:
                    stage = stage_pool.tile([128, 2, STAGE_ROWS, W], BF16)
                if g < 2 * N_CHUNKS and g % 2 == 0:
                    load_chunk(1, g // 2)
                conv_group(0, g, wmix0, stage)
                if g == 26:
                    wmix1 = routing_and_mix(1)

            # ---- conv image 1 ----
            for g in range(PAIRS):
                if g % gps == 0:
                    stage = stage_pool.tile([128, 2, STAGE_ROWS, W], BF16)
                conv_group(1, g, wmix1, stage, last=g >= 24)
    nc.compile()
    return nc


_NC_CACHE = {}


def _get_nc():
    if "nc" not in _NC_CACHE:
        _NC_CACHE["nc"] = build_nc()
    return _NC_CACHE["nc"]


def _prep_shared(weight, fc_w, fc_b):
    # [E, O, I, KH, KW] -> [I, E, KH, KW, O] -> [64, E*9*64], dup halves
    wt = np.ascontiguousarray(weight.transpose(2, 0, 3, 4, 1)).reshape(
        C_IN, E * NTAP * C_OUT)
    wt = np.concatenate([wt, wt], axis=0).astype(BF16_NP)
    fcw = np.concatenate([fc_w.T, fc_w.T], axis=0).astype(np.float32)
    fcb = np.tile(fc_b.reshape(1, E), (128, 1)).astype(np.float32)
    ones = np.ones((128, 128), np.float32)
    return wt, fcw, fcb, ones


def _prep_x(inputs):
    # pad to 258x258 zero ring, cast bf16, stack halves -> [B, 128, 130, 258]
    B = inputs.shape[0]
    xp = np.zeros((B, C_IN, H + 2, W + 2), dtype=BF16_NP)
    xp[:, :, 1:H + 1, 1:W + 1] = inputs.astype(BF16_NP)
    xh = np.empty((B, 2, C_IN, LINES, WP), dtype=BF16_NP)
    xh[:, 0] = xp[:, :, 0:LINES, :]
    xh[:, 1] = xp[:, :, H - LINES + 2:H + 2, :]
    return xh.reshape(B, 2 * C_IN, LINES, WP)


def _unpack_y(y2):
    # y2: [IMGS, 128, 128, W] quarter-permuted -> [IMGS, C_OUT, H, W] f32
    out = np.empty((y2.shape[0], C_OUT, H, W), dtype=np.float32)
    out[:, :, 0:64] = y2[:, 0:64, 0:64]
    out[:, :, 64:128] = y2[:, 64:128, 64:128]
    out[:, :, 128:192] = y2[:, 64:128, 0:64]
    out[:, :, 192:256] = y2[:, 0:64, 64:128]
    return out


def kernel(inputs, weight, fc_w, fc_b, stride=1, dilation=1, padding=1,
           _trace=False):
    assert int(stride) == 1 and int(dilation) == 1 and int(padding) == 1
    inputs = np.asarray(inputs, dtype=np.float32)
    B = inputs.shape[0]
    assert B == N_CORES * IMGS_PER_CORE
    wt, fcw, fcb, ones = _prep_shared(np.asarray(weight), np.asarray(fc_w),
                                      np.asarray(fc_b))
    xh = _prep_x(inputs)
    nc = _get_nc()
    in_maps = []
    for c in range(N_CORES):
        in_maps.append({
            "x": np.ascontiguousarray(xh[2 * c:2 * c + 2]),
            "wt": wt, "fcw": fcw, "fcb": fcb, "ones": ones,
        })
    res = run_bass_kernel_spmd(nc, in_maps, core_ids=list(range(N_CORES)),
                               trace=_trace)
    out = np.concatenate(
        [_unpack_y(res.results[c]["y"]) for c in range(N_CORES)], axis=0)
    if _trace:
        return out, res
    return out
